# revision 26
# baseline (speedup 1.0000x reference)
"""Trainium2 Bass kernel for nn_DisARM (point-proposal anchor weighting net).

Strategy (data-parallel, one batch element per NeuronCore, 8 cores):
  The whole network is a per-(point, anchor) MLP followed by a softmax +
  min-max normalization over the 64 anchors of each point.

  Per core, the 1024*64 = 65536 (point, anchor) columns stream through the
  TensorEngine in 512-column tiles.  All BatchNorm scales/biases and the
  feat_dis / spa_dis / concat / Wa1 layers are folded on the host into:
    - Wf1' (128->64, feeds tanh)                       [f1 matmul]
    - a single 124x57 block matrix S that simultaneously computes
        s1_pre  = Ws1' @ loc          (3 -> 8)
        s2_pre  = Ws2' @ s1           (8 -> 16)
        agg_pre = Wc_f @ h + Wc_s @ s2 (64+16 -> 32)
        out_pre = Wa2 @ a             (32 -> 1)
      in ONE matmul per column tile   [packed matmul]
  Wf1' lives in PE-array columns 0-63 and S in columns 64-120, so both
  stationary operands stay resident with no per-tile weight reloads.

  Layer chaining is software-pipelined through a wide fp16 SBUF "staging"
  buffer [124 partitions x W_STG]: rows 0-119 receive tanh(psum) from a
  single fused ScalarE activation per 4-tile group (h, s1, s2, a and
  t=tanh(out) all at once, with per-partition folded biases), row 120 is t,
  rows 121-123 hold the transposed locations (DMA'd in per-group pieces —
  engine APs must start at partition 0/32/64/96, DMA APs are exempt).
  Each activation writes SK=8 tiles ahead of where its group's matmuls
  read, so ScalarE (the roofline engine) streams gap-free while TensorE
  works two groups ahead; pipeline depth is 4 stages x 8 = 32 tiles,
  N_MM = 128 + 24 flush iterations = 38 groups.

  The single-partition t row is progressively redistributed by 4-tile
  SBUF-to-SBUF DMAs (1 source partition -> 16 dest partitions) into four
  [128 partitions x 128 cols] finale tiles (= 256 points x 64 anchors
  each, 2 points per partition).  The finale (softmax over anchors +
  min-max normalize) is computed directly on e = exp(t):
      w  = e / S,   wn = (1+1e-6) (e - min e) / (max e - min e + 1e-6 S)
  which is algebraically identical to normalizing w.  Pieces 0-2 are
  computed while the pipeline still runs; only piece 3 (plus one 4-tile t
  extract) trails the final activation.  Outputs stream per-piece as
  contiguous 64 KiB DMAs.
"""

import ml_dtypes
import numpy as np

BZ, NUM, NA, FD = 8, 1024, 64, 128
BN_EPS = 1e-5
N = NUM * NA          # 65536 columns per core
T = 512               # columns per matmul tile
G = 4                 # tiles per activation group
SK = 2 * G            # stage skew in tiles: act of group g feeds group g+2,
                      # so ScalarE runs concurrently with TensorE
NT = N // T           # 128 data tiles
N_MM = NT + 3 * SK    # 152 matmul iterations (pipeline flush)
N_GRP = N_MM // G     # 38 groups
W_STG = T * (N_MM + SK)  # host-side loc padding width
CHUNK = 2048          # feature DMA chunk (4 tiles, 512 KiB fp16)

NPIECE = 4            # finale pieces
PP = N // NPIECE      # 16384 t values per piece = 256 points
V = PP // (128 * NA)  # points per partition in a piece (= 2)

_CACHE = {}


def _build_bass():
    """Build the Bass/Tile graph (shapes are static; one graph for all cores)."""
    from contextlib import ExitStack

    import concourse.bacc as bacc
    import concourse.mybir as mybir
    import concourse.tile as tile

    f16 = mybir.dt.float16
    f32 = mybir.dt.float32
    f8 = mybir.dt.float8e4
    Tanh = mybir.ActivationFunctionType.Tanh
    Exp = mybir.ActivationFunctionType.Exp
    AX = mybir.AxisListType.X

    nc = bacc.Bacc()

    featD = nc.dram_tensor("feat", [FD, N], f8, kind="ExternalInput")
    locD = nc.dram_tensor("loc", [3, W_STG], f16, kind="ExternalInput")
    wf1D = nc.dram_tensor("wf1t", [FD, 64], f16, kind="ExternalInput")
    smatD = nc.dram_tensor("smat", [124, 57], f16, kind="ExternalInput")
    biasD = nc.dram_tensor("biasv", [121, 1], f32, kind="ExternalInput")
    wOutD = nc.dram_tensor("w_out", [NUM, NA], f32, kind="ExternalOutput")
    wnOutD = nc.dram_tensor("wn_out", [NUM, NA], f32, kind="ExternalOutput")

    with ExitStack() as ctx:
        tc = ctx.enter_context(tile.TileContext(nc))
        const = ctx.enter_context(tc.tile_pool(name="const", bufs=1))
        stg_pool = ctx.enter_context(tc.tile_pool(name="stg", bufs=1))
        feat_pool = ctx.enter_context(tc.tile_pool(name="featp", bufs=7))
        psum_pool = ctx.enter_context(tc.tile_pool(name="ps", bufs=2, space="PSUM"))
        fin = ctx.enter_context(tc.tile_pool(name="fin", bufs=1))

        # Staging rows: 0-63 h | 64-71 s1 | 72-87 s2 | 88-119 a | 120 t |
        # 121-123 loc.  Engine accesses must start at partition 0/32/64/96,
        # so the activation dest is stg[0:121] (identity map from psum rows)
        # and the loc rows (DMA-only) live at the top.
        stg = stg_pool.tile([124, W_STG], f16, tag="stg")

        # loc lands on only 3 SBUF partitions (~1 AXI port), so one big DMA
        # would hog 3 SDMA engines for ~18us and stall everything queued
        # behind it.  Split it into two-group pieces issued in consumption
        # order on the gpsimd SWDGE queue (that queue only emits ~1
        # trigger/us, so per-group pieces would starve the S matmuls).
        # loc is host-padded with zeros to W_STG for the flush iterations.
        LOCG = 2 * G * T
        N_LOC = N_MM // (2 * G)

        def loc_piece(p):
            sl = slice(p * LOCG, (p + 1) * LOCG)
            nc.gpsimd.dma_start(out=stg[121:124, sl], in_=locD[:, sl])

        feat_chunks = {}
        wf1 = const.tile([FD, 64], f16, tag="wf1")
        smat = const.tile([124, 57], f16, tag="smat")
        biasv = const.tile([121, 1], f32, tag="biasv")
        warm = const.tile([128, 256], f16, tag="warm")

        def chunk(c):
            if c not in feat_chunks:
                t_ = feat_pool.tile([FD, CHUNK], f8, tag="featc")
                # All feature chunks on HWDGE (sync): one ring still fans out
                # across all 16 SDMA engines, and it avoids the SWDGE Q7
                # emission jitter; gpsimd carries only loc pieces + outputs.
                # The first chunks are split per tile so the first matmuls
                # start ASAP (subtile deps: each waits only on its slice).
                eng = nc.sync
                nsplit = 2 if c == 0 else 1
                step = CHUNK // nsplit
                for s in range(nsplit):
                    lo = c * CHUNK + s * step
                    eng.dma_start(
                        out=t_[:, s * step : (s + 1) * step],
                        in_=featD[:, lo : lo + step],
                    )
                feat_chunks[c] = t_
            return feat_chunks[c]

        N_CHUNK = N // CHUNK

        # Startup: spread independent work across engine queues so nothing
        # serializes behind the ~600ns-per-trigger DMA dispatch cost.
        #   sync:   wf1 + smat, feature chunks (chunks 0/1 split per tile)
        #   scalar: biasv trigger, then the tanh/exp table load
        #   vector: memset of stg head half A
        #   gpsimd: warm memset, loc piece 0, stg head half B, loc piece 1
        # Head: the first SK tiles of the layer rows are read by group 0/1
        # matmuls before any act writes them — they only need to be FINITE.
        nc.sync.dma_start(out=wf1[:, :], in_=wf1D[:, :])
        nc.sync.dma_start(out=smat[:, :], in_=smatD[:, :])
        nc.scalar.dma_start(out=biasv[:, :], in_=biasD[:, :])
        nc.vector.memset(stg[0:121, 0 : 4 * T], 0.0)
        nc.gpsimd.memset(warm[:, :], 0.0)
        loc_piece(0)
        nc.gpsimd.memset(stg[0:121, 4 * T : SK * T], 0.0)
        loc_piece(1)
        loc_piece(2)
        chunk(0)

        # PE HAM warm-up: the PE clock-gate needs ~3.4us of sustained
        # activity to reach 2.4 GHz.  Self-contained 256-col dummy matmuls
        # on the memset warm tile keep the PE busy (and the ramp counter
        # running) from the preamble barrier until the first feature/weight
        # DMAs land at ~11us, so group 0 runs at high clock.
        ps_warm = psum_pool.tile([128, G * T], f32, tag="ps")
        for _ in range(12):
            nc.tensor.matmul(
                ps_warm[0:64, 0:256], warm[:, 0:64], warm[:, :], start=True, stop=True
            )

        for c in range(1, 7):
            chunk(c)

        # ---- finale: softmax over anchors + min-max normalization ----
        # t(d) for data tile d sits at stg[120, T*(d+4*SK) : ...].  Finale
        # piece q = points [256q, 256q+256), 2 points per partition:
        # wpiece[p, v*64+a] = t[point 256q+2p+v, anchor a].  Each 8-tile
        # extract redistributes stg[120] into 32 partitions of a wpiece by
        # SBUF-to-SBUF DMA as soon as the producing acts land (the single-
        # partition source is port-limited, ~0.9us per extract).  Extracts
        # ride the gpsimd SWDGE queue: its semaphore pool is separate from
        # the sync HWDGE ring's, which the feature chunks need to
        # themselves (sem-reuse waits there gate trigger emission on old
        # completions).
        TEX = 8
        N_TEX = NT // TEX
        EX_PP = TEX * T // 128  # dest partitions per extract (= 16)

        wpieces = []
        for q in range(NPIECE):
            wp = fin.tile([128, PP // 128], f16, tag=f"wp{q}", name=f"wp{q}")
            wpieces.append(wp)

        def t_extract(k):
            c0 = k * TEX * T
            q, p0 = k * TEX * T // PP, (k * TEX * T % PP) // (PP // 128)
            nc.gpsimd.dma_start(
                out=wpieces[q][p0 : p0 + EX_PP, :],
                in_=stg[120:121, 4 * SK * T + c0 : 4 * SK * T + c0 + TEX * T],
            )

        def t_ready_group(k):
            return ((k + 1) * TEX - 1 + 3 * SK) // G + 1

        # The math runs on e = exp(t):  w = e/S,
        # wn = (1+eps)(e-mn)/(mx-mn+eps*S), which equals the reference's
        # min-max normalization of w = softmax(t).
        pe = fin.tile([128, PP // 128], f32, tag="pe")
        w32 = fin.tile([128, PP // 128], f32, tag="w32")
        wn32 = fin.tile([128, PP // 128], f32, tag="wn32")
        ss = fin.tile([128, V], f32, tag="ss")
        rs = fin.tile([128, V], f32, tag="rs")
        mn = fin.tile([128, V], f32, tag="mn")
        mx = fin.tile([128, V], f32, tag="mx")
        dd = fin.tile([128, V], f32, tag="dd")
        s6 = fin.tile([128, V], f32, tag="s6")
        rk = fin.tile([128, V], f32, tag="rk")

        def piece_compute(q):
            # exp on ScalarE is issued ahead of this group's tanh act; its
            # extracts landed >=2 group-periods ago so the strict-FIFO
            # ScalarE queue never stalls on it.
            nc.scalar.activation(out=pe[:, :], in_=wpieces[q][:, :], func=Exp)
            e3 = pe[:, :].rearrange("p (g a) -> p g a", a=NA)
            nc.vector.reduce_sum(out=ss[:, :], in_=e3, axis=AX)
            nc.vector.tensor_reduce(
                out=mn[:, :], in_=e3, axis=AX, op=mybir.AluOpType.min
            )
            nc.vector.tensor_reduce(
                out=mx[:, :], in_=e3, axis=AX, op=mybir.AluOpType.max
            )
            nc.vector.reciprocal(out=rs[:, :], in_=ss[:, :])
            w3 = w32[:, :].rearrange("p (g a) -> p g a", a=NA)
            nc.vector.tensor_mul(w3, e3, rs[:, :].broadcast_to((128, V, NA)))
            nc.vector.tensor_sub(dd[:, :], mx[:, :], mn[:, :])
            nc.vector.tensor_scalar_mul(s6[:, :], ss[:, :], 1e-6)
            nc.vector.tensor_add(dd[:, :], dd[:, :], s6[:, :])
            nc.vector.reciprocal(rk[:, :], dd[:, :])
            nc.vector.tensor_scalar_mul(rk[:, :], rk[:, :], 1.0 + 1e-6)
            wn3 = wn32[:, :].rearrange("p (g a) -> p g a", a=NA)
            nc.vector.tensor_sub(wn3, e3, mn[:, :].broadcast_to((128, V, NA)))
            nc.vector.tensor_mul(wn3, wn3, rk[:, :].broadcast_to((128, V, NA)))
            # Outputs ride the gpsimd SWDGE queue so their 256 KiB bursts
            # never queue ahead of feature chunks on the sync HWDGE ring.
            rows = slice(q * 2 * 128, (q + 1) * 2 * 128)
            nc.gpsimd.dma_start(
                out=wOutD[rows, :].rearrange("(p v) a -> p (v a)", v=V),
                in_=w32[:, :],
            )
            nc.gpsimd.dma_start(
                out=wnOutD[rows, :].rearrange("(p v) a -> p (v a)", v=V),
                in_=wn32[:, :],
            )

        # Piece q's last extract is k = 4q+3, issued at group 8q+14; the
        # compute runs three groups later (FIFO safety margin).
        for g in range(N_GRP):
            # Prefetch: chunk c feeds group c+2; trigger 6 groups ahead so
            # the ~2us chunk transfer never gates the pipeline.
            chunk(min(g + 4, N_CHUNK - 1))
            # loc piece p covers groups [2p, 2p+2); keep ~3 pieces in flight.
            if g % 2 == 0 and g // 2 + 2 < N_LOC:
                loc_piece(g // 2 + 2)
            for k in range(N_TEX):
                if g == t_ready_group(k):
                    t_extract(k)
            for q in range(NPIECE - 1):
                if g == 8 * q + 17:
                    piece_compute(q)
            ps = psum_pool.tile([128, G * T], f32, tag="ps")
            # Interleave f1 and packed matmuls: adjacent pairs target
            # disjoint PE column groups (0-63 vs 64-120), hiding LDWEIGHTS
            # behind the previous matmul.  Flush iterations (i-SK > NT-1)
            # skip f1: the h rows they feed belong to out-of-range lineage,
            # and stale psum values are finite, so the fused act can read
            # them harmlessly.
            for k in range(G):
                i = g * G + k
                if i - SK <= NT - 1:
                    ft = max(i - SK, 0)
                    ck = chunk(ft // (CHUNK // T))
                    sl = ft % (CHUNK // T)
                    nc.tensor.matmul(
                        ps[0:64, k * T : (k + 1) * T],
                        wf1[:, :],
                        ck[:, sl * T : (sl + 1) * T],
                        start=True,
                        stop=True,
                    )
                nc.tensor.matmul(
                    ps[64:121, k * T : (k + 1) * T],
                    smat[:, :],
                    stg[0:124, i * T : (i + 1) * T],
                    start=True,
                    stop=True,
                )
            # One fused tanh for h/s1/s2/a/t of the whole group, written SK
            # tiles ahead of where this group's matmuls read.
            nc.scalar.activation(
                out=stg[0:121, (g * G + SK) * T : (g * G + SK + G) * T],
                in_=ps[0:121, :],
                func=Tanh,
                bias=biasv[:, 0:1],
                scale=1.0,
            )

        for k in range(N_TEX):
            if t_ready_group(k) >= N_GRP:
                t_extract(k)
        piece_compute(NPIECE - 1)

    nc.compile()
    return nc


def _fold_weights(inputs):
    """Fold BN + layer compositions into Wf1'/S/bias on the host (float64)."""
    W = {k: np.asarray(v, dtype=np.float64) for k, v in inputs.items()
         if k not in ("locations", "features")}
    sf1 = W["gf1"] / np.sqrt(1.0 + BN_EPS)
    W1 = W["Wf1"] * sf1[:, None]
    b1 = W["bf1"] * sf1 + W["btf1"]
    ss1 = W["gs1"] / np.sqrt(1.0 + BN_EPS)
    Ws1p = W["Ws1"] * ss1[:, None]
    bs1p = W["bs1"] * ss1 + W["bts1"]
    ss2 = W["gs2"] / np.sqrt(1.0 + BN_EPS)
    Ws2p = W["Ws2"] * ss2[:, None]
    bs2p = W["bs2"] * ss2 + W["bts2"]
    sa1 = W["ga1"] / np.sqrt(1.0 + BN_EPS)
    Wa1s, Wa1f = W["Wa1"][:, :32], W["Wa1"][:, 32:]
    Wc_s = sa1[:, None] * (Wa1s @ W["Ws3"])      # (32, 16)
    Wc_f = sa1[:, None] * (Wa1f @ W["Wf2"])      # (32, 64)
    bc = sa1 * (Wa1s @ W["bs3"] + Wa1f @ W["bf2"] + W["ba1"]) + W["bta1"]
    Wa2, ba2 = W["Wa2"], W["ba2"]

    # Block matrix S [124 K-rows, 57 M-cols]; staging rows:
    #   0-63 h | 64-71 s1 | 72-87 s2 | 88-119 a | 120 t | 121-123 loc
    # psum rows (packed matmul M at col offset 64):
    #   64-71 s1_pre | 72-87 s2_pre | 88-119 agg_pre | 120 out_pre
    S = np.zeros((124, 57), np.float64)
    S[0:64, 24:56] = Wc_f.T         # h -> agg_pre
    S[64:72, 8:24] = Ws2p.T         # s1 -> s2_pre
    S[72:88, 24:56] = Wc_s.T        # s2 -> agg_pre
    S[88:120, 56:57] = Wa2.T        # a -> out_pre
    S[121:124, 0:8] = Ws1p.T        # loc -> s1_pre

    bias = np.zeros((121, 1), np.float32)
    bias[0:64, 0] = b1
    bias[64:72, 0] = bs1p
    bias[72:88, 0] = bs2p
    bias[88:120, 0] = bc
    bias[120, 0] = ba2[0]

    return (
        np.ascontiguousarray(W1.T.astype(np.float16)),   # [128, 64] lhsT
        np.ascontiguousarray(S.astype(np.float16)),      # [124, 57] lhsT
        bias,                                            # [121, 1] f32
    )


def _ensure_axon_hooks_importable():
    """bass_utils imports antenv.axon_hooks when tracing is requested (e.g.
    via a stray BASS_TRACE env var); provide a null shim if it's missing so
    execution degrades to no-trace instead of crashing."""
    try:
        import antenv.axon_hooks  # noqa: F401
    except ImportError:
        import sys
        import types

        import antenv

        mod = types.ModuleType("antenv.axon_hooks")
        _state = {"h": None}
        mod.set_axon_ntff_profile_hook = lambda h: _state.__setitem__("h", h)
        mod.get_axon_ntff_profile_hook = lambda: _state["h"]
        sys.modules["antenv.axon_hooks"] = mod
        antenv.axon_hooks = mod


def _run(inputs, trace=False):
    _ensure_axon_hooks_importable()
    from concourse.bass_utils import run_bass_kernel_spmd

    if "nc" not in _CACHE:
        _CACHE["nc"] = _build_bass()
    nc = _CACHE["nc"]

    wf1t, smat, biasv = _fold_weights(inputs)
    features = np.asarray(inputs["features"], dtype=np.float32)
    locations = np.asarray(inputs["locations"], dtype=np.float32)

    in_maps = []
    for b in range(BZ):
        feat8 = np.ascontiguousarray(
            features[b].reshape(FD, N).astype(ml_dtypes.float8_e4m3fn))
        loc16 = np.zeros((3, W_STG), np.float16)
        loc16[:, :N] = locations[b].transpose(2, 0, 1).reshape(3, N)
        in_maps.append({
            "feat": feat8,
            "loc": loc16,
            "wf1t": wf1t,
            "smat": smat,
            "biasv": biasv,
        })

    res = run_bass_kernel_spmd(nc, in_maps, core_ids=list(range(BZ)), trace=trace)

    w = np.zeros((BZ, 1, NUM, NA), np.float32)
    wn = np.zeros((BZ, 1, NUM, NA), np.float32)
    for b in range(BZ):
        w[b, 0] = res.results[b]["w_out"].reshape(NUM, NA)
        wn[b, 0] = res.results[b]["wn_out"].reshape(NUM, NA)
    return (w, wn), res


def kernel(**inputs):
    (w, wn), _ = _run(inputs, trace=False)
    return (w, wn)


# revision 27
# speedup vs baseline: 1.0479x; 1.0479x over previous
"""Trainium2 Bass kernel for nn_DisARM (point-proposal anchor weighting net).

Strategy (data-parallel, one batch element per NeuronCore, 8 cores):
  The whole network is a per-(point, anchor) MLP followed by a softmax +
  min-max normalization over the 64 anchors of each point.

  Per core, the 1024*64 = 65536 (point, anchor) columns stream through the
  TensorEngine in 512-column tiles.  All BatchNorm scales/biases and the
  feat_dis / spa_dis / concat / Wa1 layers are folded on the host into:
    - Wf1' (128->64, feeds tanh)                       [f1 matmul]
    - a single 124x57 block matrix S that simultaneously computes
        s1_pre  = Ws1' @ loc          (3 -> 8)
        s2_pre  = Ws2' @ s1           (8 -> 16)
        agg_pre = Wc_f @ h + Wc_s @ s2 (64+16 -> 32)
        out_pre = Wa2 @ a             (32 -> 1)
      in ONE matmul per column tile   [packed matmul]
  Wf1' lives in PE-array columns 0-63 and S in columns 64-120, so both
  stationary operands stay resident with no per-tile weight reloads.

  Layer chaining is software-pipelined through a wide fp16 SBUF "staging"
  buffer [124 partitions x W_STG]: rows 0-119 receive tanh(psum) from a
  single fused ScalarE activation per 4-tile group (h, s1, s2, a and
  t=tanh(out) all at once, with per-partition folded biases), row 120 is t,
  rows 121-123 hold the transposed locations (DMA'd in per-group pieces —
  engine APs must start at partition 0/32/64/96, DMA APs are exempt).
  Each activation writes SK=8 tiles ahead of where its group's matmuls
  read, so ScalarE (the roofline engine) streams gap-free while TensorE
  works two groups ahead; pipeline depth is 4 stages x 8 = 32 tiles,
  N_MM = 128 + 24 flush iterations = 38 groups.

  The single-partition t row is progressively redistributed by 4-tile
  SBUF-to-SBUF DMAs (1 source partition -> 16 dest partitions) into four
  [128 partitions x 128 cols] finale tiles (= 256 points x 64 anchors
  each, 2 points per partition).  The finale (softmax over anchors +
  min-max normalize) is computed directly on e = exp(t):
      w  = e / S,   wn = (1+1e-6) (e - min e) / (max e - min e + 1e-6 S)
  which is algebraically identical to normalizing w.  Pieces 0-2 are
  computed while the pipeline still runs; only piece 3 (plus one 4-tile t
  extract) trails the final activation.  Outputs stream per-piece as
  contiguous 64 KiB DMAs.
"""

import ml_dtypes
import numpy as np

BZ, NUM, NA, FD = 8, 1024, 64, 128
BN_EPS = 1e-5
N = NUM * NA          # 65536 columns per core
T = 512               # columns per matmul tile
G = 4                 # tiles per activation group
SK = 2 * G            # stage skew in tiles: act of group g feeds group g+2,
                      # so ScalarE runs concurrently with TensorE
NT = N // T           # 128 data tiles
N_MM = NT + 3 * SK    # 152 matmul iterations (pipeline flush)
N_GRP = N_MM // G     # 38 groups
W_STG = T * (N_MM + SK)  # host-side loc padding width
CHUNK = 2048          # feature DMA chunk (4 tiles, 512 KiB fp16)

NPIECE = 4            # finale pieces
PP = N // NPIECE      # 16384 t values per piece = 256 points
V = PP // (128 * NA)  # points per partition in a piece (= 2)

_CACHE = {}


def _build_bass():
    """Build the Bass/Tile graph (shapes are static; one graph for all cores)."""
    from contextlib import ExitStack

    import concourse.bacc as bacc
    import concourse.mybir as mybir
    import concourse.tile as tile

    f16 = mybir.dt.float16
    f32 = mybir.dt.float32
    f8 = mybir.dt.float8e4
    Tanh = mybir.ActivationFunctionType.Tanh
    Exp = mybir.ActivationFunctionType.Exp
    AX = mybir.AxisListType.X

    nc = bacc.Bacc()

    featD = nc.dram_tensor("feat", [FD, N], f8, kind="ExternalInput")
    locD = nc.dram_tensor("loc", [3, W_STG], f16, kind="ExternalInput")
    wf1D = nc.dram_tensor("wf1t", [FD, 64], f16, kind="ExternalInput")
    smatD = nc.dram_tensor("smat", [124, 57], f16, kind="ExternalInput")
    biasD = nc.dram_tensor("biasv", [121, 1], f32, kind="ExternalInput")
    wOutD = nc.dram_tensor("w_out", [NUM, NA], f32, kind="ExternalOutput")
    wnOutD = nc.dram_tensor("wn_out", [NUM, NA], f32, kind="ExternalOutput")

    with ExitStack() as ctx:
        tc = ctx.enter_context(tile.TileContext(nc))
        const = ctx.enter_context(tc.tile_pool(name="const", bufs=1))
        stg_pool = ctx.enter_context(tc.tile_pool(name="stg", bufs=1))
        feat_pool = ctx.enter_context(tc.tile_pool(name="featp", bufs=7))
        psum_pool = ctx.enter_context(tc.tile_pool(name="ps", bufs=2, space="PSUM"))
        fin = ctx.enter_context(tc.tile_pool(name="fin", bufs=1))

        # Staging rows: 0-63 h | 64-71 s1 | 72-87 s2 | 88-119 a | 120 t |
        # 121-123 loc.  Engine accesses must start at partition 0/32/64/96,
        # so the activation dest is stg[0:121] (identity map from psum rows)
        # and the loc rows (DMA-only) live at the top.
        stg = stg_pool.tile([124, W_STG], f16, tag="stg")

        # loc lands on only 3 SBUF partitions (~1 AXI port), so one big DMA
        # would hog 3 SDMA engines for ~18us and stall everything queued
        # behind it.  Split it into two-group pieces issued in consumption
        # order on the gpsimd SWDGE queue (that queue only emits ~1
        # trigger/us, so per-group pieces would starve the S matmuls).
        # loc is host-padded with zeros to W_STG for the flush iterations.
        LOCG = 2 * G * T
        N_LOC = N_MM // (2 * G)

        def loc_piece(p):
            sl = slice(p * LOCG, (p + 1) * LOCG)
            nc.gpsimd.dma_start(out=stg[121:124, sl], in_=locD[:, sl])

        feat_chunks = {}
        wf1 = const.tile([FD, 64], f16, tag="wf1")
        smat = const.tile([124, 57], f16, tag="smat")
        biasv = const.tile([121, 1], f32, tag="biasv")
        warm = const.tile([128, 256], f16, tag="warm")

        def chunk(c):
            if c not in feat_chunks:
                t_ = feat_pool.tile([FD, CHUNK], f8, tag="featc")
                # All feature chunks on HWDGE (sync): one ring still fans out
                # across all 16 SDMA engines, and it avoids the SWDGE Q7
                # emission jitter; gpsimd carries only loc pieces + outputs.
                # The first chunks are split per tile so the first matmuls
                # start ASAP (subtile deps: each waits only on its slice).
                eng = nc.sync
                nsplit = 2 if c == 0 else 1
                step = CHUNK // nsplit
                for s in range(nsplit):
                    lo = c * CHUNK + s * step
                    eng.dma_start(
                        out=t_[:, s * step : (s + 1) * step],
                        in_=featD[:, lo : lo + step],
                    )
                feat_chunks[c] = t_
            return feat_chunks[c]

        N_CHUNK = N // CHUNK

        # Startup: spread independent work across engine queues so nothing
        # serializes behind the ~600ns-per-trigger DMA dispatch cost.
        #   sync:   wf1 + smat, feature chunks (chunks 0/1 split per tile)
        #   scalar: biasv trigger, then the tanh/exp table load
        #   vector: memset of stg head half A
        #   gpsimd: warm memset, loc piece 0, stg head half B, loc piece 1
        # Head: the first SK tiles of the layer rows are read by group 0/1
        # matmuls before any act writes them — they only need to be FINITE.
        nc.sync.dma_start(out=wf1[:, :], in_=wf1D[:, :])
        nc.sync.dma_start(out=smat[:, :], in_=smatD[:, :])
        nc.scalar.dma_start(out=biasv[:, :], in_=biasD[:, :])
        nc.vector.memset(stg[0:121, 0 : 4 * T], 0.0)
        nc.gpsimd.memset(warm[:, :], 0.0)
        loc_piece(0)
        nc.gpsimd.memset(stg[0:121, 4 * T : SK * T], 0.0)
        loc_piece(1)
        loc_piece(2)
        chunk(0)

        # PE HAM warm-up: the PE clock-gate needs ~3.4us of sustained
        # activity to reach 2.4 GHz.  Self-contained 256-col dummy matmuls
        # on the memset warm tile keep the PE busy (and the ramp counter
        # running) from the preamble barrier until the first feature/weight
        # DMAs land at ~11us, so group 0 runs at high clock.
        ps_warm = psum_pool.tile([128, G * T], f32, tag="ps")
        for _ in range(12):
            nc.tensor.matmul(
                ps_warm[0:64, 0:256], warm[:, 0:64], warm[:, :], start=True, stop=True
            )

        for c in range(1, 7):
            chunk(c)

        # ---- finale: softmax over anchors + min-max normalization ----
        # t(d) for data tile d sits at stg[120, T*(d+4*SK) : ...].  Finale
        # piece q = points [256q, 256q+256), 2 points per partition:
        # wpiece[p, v*64+a] = t[point 256q+2p+v, anchor a].  Each 8-tile
        # extract redistributes stg[120] into 32 partitions of a wpiece by
        # SBUF-to-SBUF DMA as soon as the producing acts land (the single-
        # partition source is port-limited, ~0.9us per extract).  Extracts
        # ride the gpsimd SWDGE queue: its semaphore pool is separate from
        # the sync HWDGE ring's, which the feature chunks need to
        # themselves (sem-reuse waits there gate trigger emission on old
        # completions).
        TEX = 8
        N_TEX = NT // TEX
        EX_PP = TEX * T // 128  # dest partitions per extract (= 16)

        wpieces = []
        for q in range(NPIECE):
            wp = fin.tile([128, PP // 128], f16, tag=f"wp{q}", name=f"wp{q}")
            wpieces.append(wp)

        def t_extract(k):
            c0 = k * TEX * T
            q, p0 = k * TEX * T // PP, (k * TEX * T % PP) // (PP // 128)
            nc.gpsimd.dma_start(
                out=wpieces[q][p0 : p0 + EX_PP, :],
                in_=stg[120:121, 4 * SK * T + c0 : 4 * SK * T + c0 + TEX * T],
            )

        def t_ready_group(k):
            return ((k + 1) * TEX - 1 + 3 * SK) // G + 1

        # The math runs on e = exp(t):  w = e/S,
        # wn = (1+eps)(e-mn)/(mx-mn+eps*S), which equals the reference's
        # min-max normalization of w = softmax(t).
        pe = fin.tile([128, PP // 128], f32, tag="pe")
        w32 = fin.tile([128, PP // 128], f32, tag="w32")
        wn32 = fin.tile([128, PP // 128], f32, tag="wn32")
        ss = fin.tile([128, V], f32, tag="ss")
        rs = fin.tile([128, V], f32, tag="rs")
        mn = fin.tile([128, V], f32, tag="mn")
        mx = fin.tile([128, V], f32, tag="mx")
        dd = fin.tile([128, V], f32, tag="dd")
        s6 = fin.tile([128, V], f32, tag="s6")
        rk = fin.tile([128, V], f32, tag="rk")

        def piece_compute(q):
            # exp on ScalarE is issued ahead of this group's tanh act; its
            # extracts landed >=2 group-periods ago so the strict-FIFO
            # ScalarE queue never stalls on it.
            nc.scalar.activation(out=pe[:, :], in_=wpieces[q][:, :], func=Exp)
            e3 = pe[:, :].rearrange("p (g a) -> p g a", a=NA)
            nc.vector.reduce_sum(out=ss[:, :], in_=e3, axis=AX)
            nc.vector.tensor_reduce(
                out=mn[:, :], in_=e3, axis=AX, op=mybir.AluOpType.min
            )
            nc.vector.tensor_reduce(
                out=mx[:, :], in_=e3, axis=AX, op=mybir.AluOpType.max
            )
            nc.vector.reciprocal(out=rs[:, :], in_=ss[:, :])
            w3 = w32[:, :].rearrange("p (g a) -> p g a", a=NA)
            nc.vector.tensor_mul(w3, e3, rs[:, :].broadcast_to((128, V, NA)))
            nc.vector.tensor_sub(dd[:, :], mx[:, :], mn[:, :])
            nc.vector.tensor_scalar_mul(s6[:, :], ss[:, :], 1e-6)
            nc.vector.tensor_add(dd[:, :], dd[:, :], s6[:, :])
            nc.vector.reciprocal(rk[:, :], dd[:, :])
            nc.vector.tensor_scalar_mul(rk[:, :], rk[:, :], 1.0 + 1e-6)
            wn3 = wn32[:, :].rearrange("p (g a) -> p g a", a=NA)
            nc.vector.tensor_sub(wn3, e3, mn[:, :].broadcast_to((128, V, NA)))
            nc.vector.tensor_mul(wn3, wn3, rk[:, :].broadcast_to((128, V, NA)))
            # Outputs ride the gpsimd SWDGE queue so their 256 KiB bursts
            # never queue ahead of feature chunks on the sync HWDGE ring.
            rows = slice(q * 2 * 128, (q + 1) * 2 * 128)
            nc.gpsimd.dma_start(
                out=wOutD[rows, :].rearrange("(p v) a -> p (v a)", v=V),
                in_=w32[:, :],
            )
            nc.gpsimd.dma_start(
                out=wnOutD[rows, :].rearrange("(p v) a -> p (v a)", v=V),
                in_=wn32[:, :],
            )

        # Piece q's last extract is k = 4q+3, issued at group 8q+14; the
        # compute runs three groups later (FIFO safety margin).
        for g in range(N_GRP):
            # Prefetch: chunk c feeds group c+2; trigger 6 groups ahead so
            # the ~2us chunk transfer never gates the pipeline.
            chunk(min(g + 4, N_CHUNK - 1))
            # loc piece p covers groups [2p, 2p+2); pieces 0-2 are issued at
            # startup, so emit p = g//2+3 here for a ~6-group lead.
            if g % 2 == 0 and g // 2 + 3 < N_LOC:
                loc_piece(g // 2 + 3)
            for k in range(N_TEX):
                if g == t_ready_group(k):
                    t_extract(k)
            for q in range(NPIECE - 1):
                if g == 8 * q + 17:
                    piece_compute(q)
            ps = psum_pool.tile([128, G * T], f32, tag="ps")
            # Interleave f1 and packed matmuls: adjacent pairs target
            # disjoint PE column groups (0-63 vs 64-120), hiding LDWEIGHTS
            # behind the previous matmul.  Flush iterations (i-SK > NT-1)
            # skip f1: the h rows they feed belong to out-of-range lineage,
            # and stale psum values are finite, so the fused act can read
            # them harmlessly.
            for k in range(G):
                i = g * G + k
                if i - SK <= NT - 1:
                    ft = max(i - SK, 0)
                    ck = chunk(ft // (CHUNK // T))
                    sl = ft % (CHUNK // T)
                    nc.tensor.matmul(
                        ps[0:64, k * T : (k + 1) * T],
                        wf1[:, :],
                        ck[:, sl * T : (sl + 1) * T],
                        start=True,
                        stop=True,
                    )
                nc.tensor.matmul(
                    ps[64:121, k * T : (k + 1) * T],
                    smat[:, :],
                    stg[0:124, i * T : (i + 1) * T],
                    start=True,
                    stop=True,
                )
            # One fused tanh for h/s1/s2/a/t of the whole group, written SK
            # tiles ahead of where this group's matmuls read.
            nc.scalar.activation(
                out=stg[0:121, (g * G + SK) * T : (g * G + SK + G) * T],
                in_=ps[0:121, :],
                func=Tanh,
                bias=biasv[:, 0:1],
                scale=1.0,
            )

        for k in range(N_TEX):
            if t_ready_group(k) >= N_GRP:
                t_extract(k)
        piece_compute(NPIECE - 1)

    nc.compile()
    return nc


def _fold_weights(inputs):
    """Fold BN + layer compositions into Wf1'/S/bias on the host (float64)."""
    W = {k: np.asarray(v, dtype=np.float64) for k, v in inputs.items()
         if k not in ("locations", "features")}
    sf1 = W["gf1"] / np.sqrt(1.0 + BN_EPS)
    W1 = W["Wf1"] * sf1[:, None]
    b1 = W["bf1"] * sf1 + W["btf1"]
    ss1 = W["gs1"] / np.sqrt(1.0 + BN_EPS)
    Ws1p = W["Ws1"] * ss1[:, None]
    bs1p = W["bs1"] * ss1 + W["bts1"]
    ss2 = W["gs2"] / np.sqrt(1.0 + BN_EPS)
    Ws2p = W["Ws2"] * ss2[:, None]
    bs2p = W["bs2"] * ss2 + W["bts2"]
    sa1 = W["ga1"] / np.sqrt(1.0 + BN_EPS)
    Wa1s, Wa1f = W["Wa1"][:, :32], W["Wa1"][:, 32:]
    Wc_s = sa1[:, None] * (Wa1s @ W["Ws3"])      # (32, 16)
    Wc_f = sa1[:, None] * (Wa1f @ W["Wf2"])      # (32, 64)
    bc = sa1 * (Wa1s @ W["bs3"] + Wa1f @ W["bf2"] + W["ba1"]) + W["bta1"]
    Wa2, ba2 = W["Wa2"], W["ba2"]

    # Block matrix S [124 K-rows, 57 M-cols]; staging rows:
    #   0-63 h | 64-71 s1 | 72-87 s2 | 88-119 a | 120 t | 121-123 loc
    # psum rows (packed matmul M at col offset 64):
    #   64-71 s1_pre | 72-87 s2_pre | 88-119 agg_pre | 120 out_pre
    S = np.zeros((124, 57), np.float64)
    S[0:64, 24:56] = Wc_f.T         # h -> agg_pre
    S[64:72, 8:24] = Ws2p.T         # s1 -> s2_pre
    S[72:88, 24:56] = Wc_s.T        # s2 -> agg_pre
    S[88:120, 56:57] = Wa2.T        # a -> out_pre
    S[121:124, 0:8] = Ws1p.T        # loc -> s1_pre

    bias = np.zeros((121, 1), np.float32)
    bias[0:64, 0] = b1
    bias[64:72, 0] = bs1p
    bias[72:88, 0] = bs2p
    bias[88:120, 0] = bc
    bias[120, 0] = ba2[0]

    return (
        np.ascontiguousarray(W1.T.astype(np.float16)),   # [128, 64] lhsT
        np.ascontiguousarray(S.astype(np.float16)),      # [124, 57] lhsT
        bias,                                            # [121, 1] f32
    )


def _ensure_axon_hooks_importable():
    """bass_utils imports antenv.axon_hooks when tracing is requested (e.g.
    via a stray BASS_TRACE env var); provide a null shim if it's missing so
    execution degrades to no-trace instead of crashing."""
    try:
        import antenv.axon_hooks  # noqa: F401
    except ImportError:
        import sys
        import types

        import antenv

        mod = types.ModuleType("antenv.axon_hooks")
        _state = {"h": None}
        mod.set_axon_ntff_profile_hook = lambda h: _state.__setitem__("h", h)
        mod.get_axon_ntff_profile_hook = lambda: _state["h"]
        sys.modules["antenv.axon_hooks"] = mod
        antenv.axon_hooks = mod


def _run(inputs, trace=False):
    _ensure_axon_hooks_importable()
    from concourse.bass_utils import run_bass_kernel_spmd

    if "nc" not in _CACHE:
        _CACHE["nc"] = _build_bass()
    nc = _CACHE["nc"]

    wf1t, smat, biasv = _fold_weights(inputs)
    features = np.asarray(inputs["features"], dtype=np.float32)
    locations = np.asarray(inputs["locations"], dtype=np.float32)

    in_maps = []
    for b in range(BZ):
        feat8 = np.ascontiguousarray(
            features[b].reshape(FD, N).astype(ml_dtypes.float8_e4m3fn))
        loc16 = np.zeros((3, W_STG), np.float16)
        loc16[:, :N] = locations[b].transpose(2, 0, 1).reshape(3, N)
        in_maps.append({
            "feat": feat8,
            "loc": loc16,
            "wf1t": wf1t,
            "smat": smat,
            "biasv": biasv,
        })

    res = run_bass_kernel_spmd(nc, in_maps, core_ids=list(range(BZ)), trace=trace)

    w = np.zeros((BZ, 1, NUM, NA), np.float32)
    wn = np.zeros((BZ, 1, NUM, NA), np.float32)
    for b in range(BZ):
        w[b, 0] = res.results[b]["w_out"].reshape(NUM, NA)
        wn[b, 0] = res.results[b]["wn_out"].reshape(NUM, NA)
    return (w, wn), res


def kernel(**inputs):
    (w, wn), _ = _run(inputs, trace=False)
    return (w, wn)
